# revision 1
# baseline (speedup 1.0000x reference)
"""Trainium2 Bass kernel for nn_KernelConv_80668075753604 (gnn_message_passing).

Strategy
--------
All scores reduce to distances of the form  d = |a_n - b_m|^2  between per-node
vectors a_n and per-(l,perm) table vectors b_m (m = l*24+p, M=768 columns):

  d[n,m] = |a_n|^2 + |b_m|^2 - 2 <a_n, b_m>

computed with PE matmuls (contraction over the feature dim).  The |b|^2 (and,
where K allows, |a|^2) terms are folded into the matmul itself via extra
ones-rows in the stationary operand.  Key identity: atan(1/d) - pi/2 =
-atan(d), so (score - pi/2)^2 = atan(d)^2 and no atan is ever needed on the
[L,P,N]-sized tensors — only on the [L,N] reduced ones.  The argmax over
permutations of atan(1/d) equals the argmin over p of d (first index on ties),
implemented as min-reduce + is_equal + ramp-weighted first-match one-hot, and
the one-hot performs the "best permutation" gathers for the edge/angle/length
scores as multiply + segmented reduce.

Sharding: N=50000 focal nodes split across 8 cores (6250 -> padded 6272 = 49
tiles of 128 nodes); the tiny [L,P,*] permutation tables are replicated.
"""

import math
import os
import sys
from itertools import permutations

import numpy as np

for _p in ("/opt/trn_rl_repo",):
    if _p not in sys.path and os.path.isdir(_p):
        sys.path.insert(0, _p)

import concourse.bass as bass
import concourse.tile as tile
from concourse import bacc, mybir
from concourse.bass_utils import run_bass_kernel_spmd
from concourse.masks import make_identity

AF = mybir.ActivationFunctionType
ALU = mybir.AluOpType
AX = mybir.AxisListType
DT = mybir.dt.float32

S, NPERM, L, F, E, D = 4, 24, 32, 32, 16, 3
M = L * NPERM                       # 768
SF, SE, SD = S * F, S * E, S * D    # 128, 64, 12
NCORES = 8
N_FULL = 50000
N_CORE = N_FULL // NCORES           # 6250
TILE = 128
NTILES_FULL = (N_CORE + TILE - 1) // TILE   # 49
PKW = SF + SE + 1 + F + SD + D      # 240
# packed column layout
C_XN, C_ED, C_ONE, C_XF, C_PN, C_PF = 0, 128, 192, 193, 225, 237
HALF_PI = float(np.float32(math.pi / 2))
EPS = 1e-8

PERMS = np.array(list(permutations(range(S))), dtype=np.int64)  # [24, 4]


def _bcast_ap(handle, parts=128):
    ap = handle[:]
    return bass.AP(tensor=ap.tensor, offset=ap.offset, ap=[[0, parts]] + list(ap.ap))


def build_nc(ntiles=NTILES_FULL):
    nc = bacc.Bacc("TRN2")
    npad = ntiles * TILE
    pk = nc.declare_dram_parameter("pk", [npad, PKW], DT, isOutput=False)
    w_x = nc.declare_dram_parameter("w_x", [SF, M], DT, isOutput=False)
    w_sqx = nc.declare_dram_parameter("w_sqx", [M], DT, isOutput=False)
    w_e = nc.declare_dram_parameter("w_e", [SE + 1, M], DT, isOutput=False)
    w_al = nc.declare_dram_parameter("w_al", [11, 2 * M], DT, isOutput=False)
    w_c = nc.declare_dram_parameter("w_c", [F + 1, L], DT, isOutput=False)
    w_ramp = nc.declare_dram_parameter("w_ramp", [M], DT, isOutput=False)
    out = nc.declare_dram_parameter("out", [L, npad], DT, isOutput=True)

    with tile.TileContext(nc) as tc:
        with (
            tc.tile_pool(name="const", bufs=1) as cp,
            tc.tile_pool(name="work", bufs=3) as wp,
            tc.tile_pool(name="pmm", bufs=1, space="PSUM") as pmm,
            tc.tile_pool(name="palp", bufs=2, space="PSUM") as palp,
            tc.tile_pool(name="psm", bufs=2, space="PSUM") as psm,
        ):
            ident = cp.tile([128, 128], DT, tag="ident")
            make_identity(nc, ident)
            rx = cp.tile([SF, M], DT, tag="rx")
            nc.sync.dma_start(out=rx, in_=w_x[:])
            re = cp.tile([SE + 1, M], DT, tag="re")
            nc.sync.dma_start(out=re, in_=w_e[:])
            ral = cp.tile([11, 2 * M], DT, tag="ral")
            nc.sync.dma_start(out=ral, in_=w_al[:])
            rc = cp.tile([128, L], DT, tag="rc")
            nc.sync.dma_start(out=rc[64:64 + F + 1], in_=w_c[:])
            sqsx = cp.tile([128, M], DT, tag="sqsx")
            nc.sync.dma_start(out=sqsx, in_=_bcast_ap(w_sqx))
            ramp = cp.tile([128, M], DT, tag="ramp")
            nc.sync.dma_start(out=ramp, in_=_bcast_ap(w_ramp))
            hpi = cp.tile([128, 1], DT, tag="hpi")
            nc.vector.memset(hpi, HALF_PI)

            for t in range(ntiles):
                r0 = t * TILE
                pk_t = wp.tile([128, PKW], DT, tag="pk")
                nc.sync.dma_start(out=pk_t, in_=pk[r0:r0 + TILE, :])

                # ---- transposes (PE) + PSUM->SBUF copies ----
                xnT_t = psm.tile([128, 128], DT, tag="tp")
                nc.tensor.transpose(xnT_t, pk_t[:, C_XN:C_XN + SF], ident)
                xnT = wp.tile([128, 128], DT, tag="xnT")
                nc.vector.tensor_copy(xnT, xnT_t)

                ecT_t = psm.tile([128, 128], DT, tag="tp")
                nc.tensor.transpose(ecT_t[0:97], pk_t[:, C_ED:C_PN], ident)
                ecT = wp.tile([128, 128], DT, tag="ecT")
                nc.vector.tensor_copy(ecT[0:97], ecT_t[0:97])

                # ---- geometry: pn_rel, intra/len (small vector ops) ----
                pn_ap = pk_t[:, C_PN:C_PN + SD].rearrange("p (s d) -> p s d", d=D)
                pf_b = pk_t[:, C_PF:C_PF + D].unsqueeze(1).broadcast_to([128, S, D])
                pnr = wp.tile([128, S, D], DT, tag="pnr")
                nc.vector.tensor_tensor(pnr, pn_ap, pf_b, op=ALU.subtract)

                prod = wp.tile([128, S, D], DT, tag="prod")
                nc.vector.tensor_mul(prod[:, 1:4, :], pnr[:, 1:4, :], pnr[:, 0:3, :])
                nc.vector.tensor_mul(prod[:, 0:1, :], pnr[:, 0:1, :], pnr[:, 3:4, :])
                dot = wp.tile([128, S], DT, tag="dot")
                nc.vector.tensor_reduce(dot, prod, axis=AX.X, op=ALU.add)

                sqp = wp.tile([128, S, D], DT, tag="sqp")
                nc.scalar.activation(sqp, pnr, AF.Square)
                norm2 = wp.tile([128, S], DT, tag="norm2")
                nc.vector.tensor_reduce(norm2, sqp, axis=AX.X, op=ALU.add)

                pk2 = wp.tile([128, 11], DT, tag="pk2")
                nc.scalar.activation(pk2[:, 4:8], norm2, AF.Sqrt)        # len_nei
                nc.vector.tensor_reduce(pk2[:, 10:11], norm2, axis=AX.X, op=ALU.add)
                nmax = wp.tile([128, S], DT, tag="nmax")
                nc.vector.tensor_single_scalar(nmax, pk2[:, 4:8], EPS, op=ALU.max)
                den = wp.tile([128, S], DT, tag="den")
                nc.vector.tensor_mul(den[:, 1:4], nmax[:, 1:4], nmax[:, 0:3])
                nc.vector.tensor_mul(den[:, 0:1], nmax[:, 0:1], nmax[:, 3:4])
                rden = wp.tile([128, S], DT, tag="rden")
                nc.vector.reciprocal(rden, den)
                nc.vector.tensor_mul(pk2[:, 0:4], dot, rden)             # intra_nei
                nc.vector.memset(pk2[:, 8:9], 1.0)
                isq = wp.tile([128, S], DT, tag="isq")
                nc.scalar.activation(isq, pk2[:, 0:4], AF.Square, accum_out=pk2[:, 9:10])

                # ---- per-node squared norms ----
                scr = wp.tile([128, 128], DT, tag="scr")
                sq_xn = wp.tile([128, 1], DT, tag="sq_xn")
                nc.scalar.activation(scr, pk_t[:, C_XN:C_XN + SF], AF.Square,
                                     accum_out=sq_xn)
                sq_e = wp.tile([128, 1], DT, tag="sq_e")
                nc.scalar.activation(scr[:, 0:SE], pk_t[:, C_ED:C_ED + SE], AF.Square,
                                     accum_out=sq_e)
                sq_xf = wp.tile([128, 1], DT, tag="sq_xf")
                nc.scalar.activation(scr[:, 64:96], pk_t[:, C_XF:C_XF + F], AF.Square,
                                     accum_out=sq_xf)

                p2T_t = psm.tile([128, 128], DT, tag="tp")
                nc.tensor.transpose(p2T_t[0:11], pk2, ident)
                p2T = wp.tile([128, 128], DT, tag="p2T")
                nc.vector.tensor_copy(p2T[0:11], p2T_t[0:11])

                # ---- matmuls ----
                px_a = pmm.tile([128, 512], DT, tag="pxa")
                nc.tensor.matmul(px_a, xnT, rx[:, 0:512], start=True, stop=True)
                px_b = pmm.tile([128, 256], DT, tag="pxb")
                nc.tensor.matmul(px_b, xnT, rx[:, 512:768], start=True, stop=True)
                pe_a = pmm.tile([128, 512], DT, tag="pea")
                nc.tensor.matmul(pe_a, ecT[0:65], re[:, 0:512], start=True, stop=True)
                pe_b = pmm.tile([128, 256], DT, tag="peb")
                nc.tensor.matmul(pe_b, ecT[0:65], re[:, 512:768], start=True, stop=True)
                pc = psm.tile([128, 32], DT, tag="tp")
                nc.tensor.matmul(pc, ecT[64:97], rc[64:97], start=True, stop=True)
                pal0 = palp.tile([128, 512], DT, tag="pal")
                nc.tensor.matmul(pal0, p2T[0:11], ral[:, 0:512], start=True, stop=True)
                pal1 = palp.tile([128, 512], DT, tag="pal")
                nc.tensor.matmul(pal1[:, 0:256], p2T[0:11], ral[:, 512:768],
                                 start=True, stop=True)
                pal2 = palp.tile([128, 512], DT, tag="pal")
                nc.tensor.matmul(pal2, p2T[0:11], ral[:, 768:1280], start=True, stop=True)
                pal3 = palp.tile([128, 512], DT, tag="pal")
                nc.tensor.matmul(pal3[:, 0:256], p2T[0:11], ral[:, 1280:1536],
                                 start=True, stop=True)

                # ---- d tensors ----
                tmpx = wp.tile([128, M], DT, tag="tmpx")
                nc.vector.tensor_add(tmpx[:, 0:512], px_a, sqsx[:, 0:512])
                nc.vector.tensor_add(tmpx[:, 512:768], px_b, sqsx[:, 512:768])
                dx = wp.tile([128, M], DT, tag="dx")
                nc.scalar.activation(dx, tmpx, AF.Relu, bias=sq_xn)
                de = wp.tile([128, M], DT, tag="de")
                nc.scalar.activation(de[:, 0:512], pe_a, AF.Relu, bias=sq_e)
                nc.scalar.activation(de[:, 512:768], pe_b, AF.Relu, bias=sq_e)
                da = wp.tile([128, M], DT, tag="da")
                nc.scalar.activation(da[:, 0:512], pal0, AF.Relu)
                nc.scalar.activation(da[:, 512:768], pal1[:, 0:256], AF.Relu)
                dl = wp.tile([128, M], DT, tag="dl")
                nc.scalar.activation(dl[:, 0:512], pal2, AF.Relu)
                nc.scalar.activation(dl[:, 512:768], pal3[:, 0:256], AF.Relu)

                dx3 = dx[:].rearrange("p (l q) -> p l q", q=NPERM)

                # ---- selection: argmin over perms, first index on ties ----
                D5 = wp.tile([128, L, 5], DT, tag="D5")
                nc.vector.tensor_reduce(D5[:, :, 0], dx3, axis=AX.X, op=ALU.min)
                eq = wp.tile([128, M], DT, tag="eq")
                eq3 = eq[:].rearrange("p (l q) -> p l q", q=NPERM)
                nc.vector.tensor_tensor(eq3, dx3, D5[:, :, 0].to_broadcast([128, L, NPERM]),
                                        op=ALU.is_equal)
                wgt = wp.tile([128, M], DT, tag="wgt")
                nc.vector.tensor_mul(wgt, eq, ramp)
                wgt3 = wgt[:].rearrange("p (l q) -> p l q", q=NPERM)
                wmax = wp.tile([128, L], DT, tag="wmax")
                nc.vector.tensor_reduce(wmax, wgt3, axis=AX.X, op=ALU.max)
                oh = wp.tile([128, M], DT, tag="oh")
                oh3 = oh[:].rearrange("p (l q) -> p l q", q=NPERM)
                nc.vector.tensor_tensor(oh3, wgt3, wmax[:].to_broadcast([128, L, NPERM]),
                                        op=ALU.is_equal)

                # ---- one-hot gathers of d_e, d_angle, d_len at best perm ----
                for k, src in ((1, de), (2, da), (3, dl)):
                    g = wp.tile([128, M], DT, tag="g")
                    nc.vector.tensor_mul(g, oh, src)
                    g3 = g[:].rearrange("p (l q) -> p l q", q=NPERM)
                    nc.vector.tensor_reduce(D5[:, :, k], g3, axis=AX.X, op=ALU.add)

                # center distance straight into D5
                nc.scalar.activation(D5[:, :, 4], pc, AF.Relu, bias=sq_xf)

                # ---- atan(d)^2 for the 5 scores;  atan via table in [0,1] ----
                f5 = D5[:].rearrange("p l k -> p (l k)")
                lo = wp.tile([128, L * 5], DT, tag="lo")
                nc.vector.tensor_single_scalar(lo, f5, 1.0, op=ALU.min)
                hi = wp.tile([128, L * 5], DT, tag="hi")
                nc.vector.tensor_single_scalar(hi, f5, 1.0, op=ALU.max)
                rcp = wp.tile([128, L * 5], DT, tag="rcp")
                nc.vector.reciprocal(rcp, hi)
                a1 = wp.tile([128, L * 5], DT, tag="a1")
                nc.scalar.activation(a1, lo, AF.Arctan)
                a2 = wp.tile([128, L * 5], DT, tag="a2")
                nc.scalar.activation(a2, rcp, AF.Arctan)
                a2p = wp.tile([128, L * 5], DT, tag="a2p")
                nc.scalar.activation(a2p, a2, AF.Identity, scale=-1.0, bias=hpi[:])
                msk = wp.tile([128, L * 5], mybir.dt.uint8, tag="msk")
                nc.vector.tensor_single_scalar(msk, f5, 1.0, op=ALU.is_le)
                atn = wp.tile([128, L * 5], DT, tag="atn")
                nc.vector.tensor_copy(atn, a2p)
                nc.vector.copy_predicated(atn, msk, a1)
                sq5 = wp.tile([128, L, 5], DT, tag="sq5")
                nc.scalar.activation(sq5[:].rearrange("p l k -> p (l k)"), atn, AF.Square)

                total = wp.tile([128, L], DT, tag="total")
                nc.vector.tensor_reduce(total, sq5, axis=AX.X, op=ALU.add)

                # ---- out = atan(1/total) = pi/2 - atan(total) ----
                lo2 = wp.tile([128, L], DT, tag="lo2")
                nc.vector.tensor_single_scalar(lo2, total, 1.0, op=ALU.min)
                hi2 = wp.tile([128, L], DT, tag="hi2")
                nc.vector.tensor_single_scalar(hi2, total, 1.0, op=ALU.max)
                rcp2 = wp.tile([128, L], DT, tag="rcp2")
                nc.vector.reciprocal(rcp2, hi2)
                b1 = wp.tile([128, L], DT, tag="b1")
                nc.scalar.activation(b1, lo2, AF.Arctan)
                o1 = wp.tile([128, L], DT, tag="o1")
                nc.scalar.activation(o1, b1, AF.Identity, scale=-1.0, bias=hpi[:])
                b2 = wp.tile([128, L], DT, tag="b2")
                nc.scalar.activation(b2, rcp2, AF.Arctan)
                msk2 = wp.tile([128, L], mybir.dt.uint8, tag="msk2")
                nc.vector.tensor_single_scalar(msk2, total, 1.0, op=ALU.is_le)
                res = wp.tile([128, L], DT, tag="res")
                nc.vector.tensor_copy(res, b2)
                nc.vector.copy_predicated(res, msk2, o1)

                resT_t = psm.tile([128, 128], DT, tag="tp")
                nc.tensor.transpose(resT_t[0:L], res, ident)
                resT = wp.tile([L, 128], DT, tag="resT")
                nc.vector.tensor_copy(resT, resT_t[0:L])
                nc.sync.dma_start(out=out[:, r0:r0 + TILE], in_=resT)
    nc.finalize()
    return nc


def _host_tables(x_support, edge_attr_support, p_support, x_center):
    f32 = np.float32
    xs = np.asarray(x_support, f32)[:, PERMS, :]          # [L,P,S,F]
    es = np.asarray(edge_attr_support, f32)[:, PERMS, :]  # [L,P,S,E]
    ps = np.asarray(p_support, f32)[:, PERMS, :]          # [L,P,S,D]
    xc = np.asarray(x_center, f32)[:, 0, :]               # [L,F]

    xs_f = xs.reshape(M, SF)
    w_x = np.ascontiguousarray((-2.0 * xs_f).T.astype(f32))
    w_sqx = (xs_f * xs_f).sum(-1).astype(f32)

    es_f = es.reshape(M, SE)
    w_e = np.empty((SE + 1, M), f32)
    w_e[0:SE] = (-2.0 * es_f).T
    w_e[SE] = (es_f * es_f).sum(-1)

    q = np.roll(ps, 1, axis=2)
    dotp = (q * ps).sum(-1)
    nq = np.maximum(np.sqrt((q * q).sum(-1)), f32(EPS))
    npn = np.maximum(np.sqrt((ps * ps).sum(-1)), f32(EPS))
    intra = (dotp / (nq * npn)).astype(f32)               # [L,P,S]
    lenp = np.sqrt((ps * ps).sum(-1)).astype(f32)         # [L,P,S]
    ia_f = intra.reshape(M, S)
    ln_f = lenp.reshape(M, S)
    w_al = np.zeros((11, 2 * M), f32)
    w_al[0:4, 0:M] = (-2.0 * ia_f).T
    w_al[4:8, M:2 * M] = (-2.0 * ln_f).T
    w_al[8, 0:M] = (ia_f * ia_f).sum(-1)
    w_al[8, M:2 * M] = (ln_f * ln_f).sum(-1)
    w_al[9, 0:M] = 1.0
    w_al[10, M:2 * M] = 1.0

    w_c = np.empty((F + 1, L), f32)
    w_c[0] = (xc * xc).sum(-1)
    w_c[1:] = (-2.0 * xc).T

    w_ramp = np.tile(np.arange(NPERM, 0, -1, dtype=f32), L)
    return dict(w_x=w_x, w_sqx=w_sqx, w_e=w_e, w_al=w_al, w_c=w_c, w_ramp=w_ramp)


def _pack_block(x_focal, p_focal, x_neighbor, p_neighbor, edge_attr_neighbor, npad):
    f32 = np.float32
    n = x_focal.shape[0]
    pk = np.ones((npad, PKW), f32)
    pk[:n, C_XN:C_XN + SF] = np.asarray(x_neighbor, f32).reshape(n, SF)
    pk[:n, C_ED:C_ED + SE] = np.asarray(edge_attr_neighbor, f32).reshape(n, SE)
    pk[:n, C_XF:C_XF + F] = np.asarray(x_focal, f32)
    pk[:n, C_PN:C_PN + SD] = np.asarray(p_neighbor, f32).reshape(n, SD)
    pk[:n, C_PF:C_PF + D] = np.asarray(p_focal, f32)
    pk[n:, C_PF:C_PF + D] = 0.0   # pads: pn_rel = 1 -> safe norms
    return pk


def _pack_nodes(x_focal, p_focal, x_neighbor, p_neighbor, edge_attr_neighbor,
                ntiles=NTILES_FULL):
    n = x_focal.shape[0]
    npad = ntiles * TILE
    per = n // NCORES
    return np.stack([
        _pack_block(x_focal[c * per:(c + 1) * per], p_focal[c * per:(c + 1) * per],
                    x_neighbor[c * per:(c + 1) * per], p_neighbor[c * per:(c + 1) * per],
                    edge_attr_neighbor[c * per:(c + 1) * per], npad)
        for c in range(NCORES)
    ])


_NC_CACHE = {}


def run_on_hw(pk, tables, ntiles=NTILES_FULL, trace=False):
    if ntiles not in _NC_CACHE:
        _NC_CACHE[ntiles] = build_nc(ntiles)
    nc = _NC_CACHE[ntiles]
    in_maps = [dict(pk=np.ascontiguousarray(pk[c]), **tables) for c in range(NCORES)]
    r = run_bass_kernel_spmd(nc, in_maps, list(range(NCORES)), trace=trace)
    return r


def kernel(**inputs):
    tables = _host_tables(inputs["x_support"], inputs["edge_attr_support"],
                          inputs["p_support"], inputs["x_center"])
    pk = _pack_nodes(inputs["x_focal"], inputs["p_focal"], inputs["x_neighbor"],
                     inputs["p_neighbor"], inputs["edge_attr_neighbor"])
    r = run_on_hw(pk, tables)
    per = N_FULL // NCORES
    out = np.concatenate([r.results[c]["out"][:, :per] for c in range(NCORES)], axis=1)
    return out.astype(np.float32)

